# revision 16
# baseline (speedup 1.0000x reference)
"""Trainium2 Bass kernel for nn_DeepTimeGraphNet (per-row conv/pool pyramid + classifier).

Contract: kernel(**inputs) takes the FULL unsharded inputs (keys as in
setup_inputs()) and returns the FULL (64, 3) softmax output.

Sharding: pure data parallel over batch. Core i handles batch rows
[8i, 8i+8) = 8192 (batch, node) rows of length 1200, processed as 8
supertiles of 1024 rows = 128 SBUF partitions x 8 column groups.

v3 design, from measured engine rates (DVE stt/TT and ScalarE ACT are all
~0.96 el/ns on fp32 regardless of stride/dtype, EXCEPT contiguous fp16
tensor_tensor which hits ~1.98 el/ns; GpSimd cannot run tensor ops):

- S1 (conv0 k2 s2): ScalarE computes both taps as separate fp16 buffers
  (ae = w00*x_even + b0, ao = w01*x_odd) via ACTIVATE at ~1 el/ns, and
  DVE combines them with one contiguous fp16 TT add at ~2 el/ns. This
  moves ~half of conv0 off the (otherwise oversubscribed) DVE.
- S2 (maxpool3+relu): fp16 TT max + fp16 stt max-max (relu fused via the
  scalar-0 slot), stride-3 reads.
- S3 (conv2 k4 s2 p1): ScalarE base tap (bias), 3 DVE fp16 stt taps.
- S4 (maxpool2+relu): one stt, fp16 srcs -> fp32 r3 staging.
- 2-deep software pipeline: iteration s emits DVE taps/S4 of s-2 and
  DVE add/pools of s-1 while ScalarE loads supertile s, so neither
  engine ever waits on the other's round trip.
- Tail (conv4..conv8, 50->1, fp32) batched over supertile groups
  [0,4) [4,6) [6,8) placed in DMA windows; PE matmul classifier +
  exact softmax.

Per-supertile engine budget: DVE ~11.5us, ScalarE ~11.4us, both under
the 12.5us DMA window (39.3 MB/core over 16 DMA engines x ~24.6 B/ns
~= 100us floor), so the stream is DMA-paced.
"""
import os
import sys

for _p in ("/root/.axon_site/_ro/trn_rl_repo", "/opt/trn_rl_repo"):
    if os.path.isdir(_p) and _p not in sys.path:
        sys.path.insert(0, _p)

import numpy as np  # noqa: E402

import concourse.bacc as bacc  # noqa: E402
import concourse.tile as tile  # noqa: E402
from concourse import mybir  # noqa: E402
from concourse.bass_utils import run_bass_kernel_spmd  # noqa: E402

F32 = mybir.dt.float32
F16 = mybir.dt.float16
Alu = mybir.AluOpType
Act = mybir.ActivationFunctionType

BS, NN, T = 64, 1024, 1200
N_CORES = 8
S_PER_CORE = 8          # supertiles per core; each = 1024 rows (one batch row)
C = 8                   # column groups per supertile (128 rows each)

_CACHE = {}


def _build(w):
    """Build + compile the per-core SPMD program with weights baked in."""
    nc = bacc.Bacc("TRN2", target_bir_lowering=False, debug=False)
    x = nc.dram_tensor("x", [S_PER_CORE * C * 128, T], F32, kind="ExternalInput")
    clswt = nc.dram_tensor("clswt", [128, 24], F32, kind="ExternalInput")
    out = nc.dram_tensor("out", [8, 3], F32, kind="ExternalOutput")

    w2, w4, w6, w8 = w["w2"], w["w4"], w["w6"], w["w8"]
    stt = nc.vector.scalar_tensor_tensor

    with tile.TileContext(nc) as tc:
        with (
            tc.tile_pool(name="xpool", bufs=9) as xpool,
            tc.tile_pool(name="aeo", bufs=12) as aeo,
            tc.tile_pool(name="wk", bufs=2) as wk,
            tc.tile_pool(name="const", bufs=1) as const,
            tc.tile_pool(name="psum", bufs=1, space="PSUM") as psum,
        ):
            clsw = const.tile([128, 24], F32)
            featmat = const.tile([128, 64], F32)

            y0 = const.tile([128, C * 600], F16)
            y0v = y0[:].rearrange("p (c t) -> p c t", c=C)
            t01 = const.tile([128, C * 200], F16)
            t01v = t01[:].rearrange("p (c t) -> p c t", c=C)

            # persistent staging for the batched small stages (fp32)
            r3all = const.tile([128, S_PER_CORE * C * 50], F32)
            y4all = const.tile([128, S_PER_CORE * C * 25], F32)
            r5all = const.tile([128, S_PER_CORE * C * 12], F32)
            y6all = const.tile([128, S_PER_CORE * C * 6], F32)
            r7all = const.tile([128, S_PER_CORE * C * 3], F32)
            fball = const.tile([128, S_PER_CORE * C], F32)
            r3v = r3all[:].rearrange("p (s c t) -> p s c t", s=S_PER_CORE, c=C)
            y4v = y4all[:].rearrange("p (s c t) -> p s c t", s=S_PER_CORE, c=C)
            r5v = r5all[:].rearrange("p (s c t) -> p s c t", s=S_PER_CORE, c=C)
            y6v = y6all[:].rearrange("p (s c t) -> p s c t", s=S_PER_CORE, c=C)
            r7v = r7all[:].rearrange("p (s c t) -> p s c t", s=S_PER_CORE, c=C)
            fbv = fball[:].rearrange("p (s c) -> p s c", s=S_PER_CORE)
            fmv = featmat[:].rearrange("p (s c) -> p s c", s=S_PER_CORE)

            x4 = x[:].rearrange("(s c p) t -> s p c t", s=S_PER_CORE, c=C, p=128)

            st = {}   # per-supertile handles

            def sca_ae_ao(s):
                """S1 taps on ScalarE: ae = w00*x_even + b0, ao = w01*x_odd
                (fp16), per quarter-supertile (fine slot recycling keeps the
                DMA queues fed and completions arriving every ~3us)."""
                d = st[s]
                d["ae"], d["ao"] = [], []
                for q in range(4):
                    xb = d["x"][q][:]
                    ae = aeo.tile([128, 1200], F16)
                    ao = aeo.tile([128, 1200], F16)
                    nc.scalar.activation(ae[:], xb[:, 0:2400:2], Act.Copy,
                                         bias=w["b0"], scale=w["w00"])
                    nc.scalar.activation(ao[:], xb[:, 1:2400:2], Act.Copy,
                                         bias=0.0, scale=w["w01"])
                    d["ae"].append(ae)
                    d["ao"].append(ao)

            def vec_add_pools(s):
                """DVE: y0 = ae + ao (contiguous fp16 TT, 2x rate), then
                maxpool3 + fused relu -> r1 (fp16)."""
                d = st[s]
                for q in range(4):
                    nc.vector.tensor_tensor(y0[:, q * 1200:(q + 1) * 1200],
                                            d["ae"][q][:], d["ao"][q][:],
                                            Alu.add)
                nc.vector.tensor_tensor(t01v, y0v[:, :, 0:600:3],
                                        y0v[:, :, 1:600:3], Alu.max)
                r1 = wk.tile([128, C * 200], F16)
                d["r1"] = r1
                stt(r1[:].rearrange("p (c t) -> p c t", c=C), t01v, 0.0,
                    y0v[:, :, 2:600:3], Alu.max, Alu.max)

            def sca_s3base(s):
                """ScalarE base tap of conv2: y2 = w2[1]*r1_even + b2."""
                d = st[s]
                y2 = wk.tile([128, C * 100], F16)
                d["y2"] = y2
                r1v = d["r1"][:].rearrange("p (c t) -> p c t", c=C)
                nc.scalar.activation(y2[:].rearrange("p (c t) -> p c t", c=C),
                                     r1v[:, :, 0:200:2], Act.Copy,
                                     bias=w["b2"], scale=w2[1])

            def vec_taps_s4(s, c0=0, cn=C):
                """DVE taps of conv2 + S4 maxpool2+relu -> fp32 r3."""
                d = st[s]
                cs = slice(c0, c0 + cn)
                r1v = d["r1"][:].rearrange("p (c t) -> p c t", c=C)[:, cs]
                y2v = d["y2"][:].rearrange("p (c t) -> p c t", c=C)[:, cs]
                stt(y2v, r1v[:, :, 1:200:2], w2[2], y2v, Alu.mult, Alu.add)
                stt(y2v[:, :, 1:100], r1v[:, :, 1:198:2], w2[0],
                    y2v[:, :, 1:100], Alu.mult, Alu.add)
                stt(y2v[:, :, 0:99], r1v[:, :, 2:199:2], w2[3],
                    y2v[:, :, 0:99], Alu.mult, Alu.add)
                stt(r3v[:, s, cs], y2v[:, :, 0:100:2], 0.0, y2v[:, :, 1:100:2],
                    Alu.max, Alu.max)

            def vec_add_q(s, q):
                d = st[s]
                nc.vector.tensor_tensor(y0[:, q * 1200:(q + 1) * 1200],
                                        d["ae"][q][:], d["ao"][q][:], Alu.add)

            def vec_pools_q(s, q):
                d = st[s]
                cs = slice(2 * q, 2 * q + 2)
                nc.vector.tensor_tensor(t01v[:, cs], y0v[:, cs, 0:600:3],
                                        y0v[:, cs, 1:600:3], Alu.max)
                stt(d["r1"][:].rearrange("p (c t) -> p c t", c=C)[:, cs],
                    t01v[:, cs], 0.0, y0v[:, cs, 2:600:3], Alu.max, Alu.max)

            def sca_s3base_q(s, q):
                d = st[s]
                cs = slice(2 * q, 2 * q + 2)
                r1v = d["r1"][:].rearrange("p (c t) -> p c t", c=C)
                y2v = d["y2"][:].rearrange("p (c t) -> p c t", c=C)
                nc.scalar.activation(y2v[:, cs], r1v[:, cs, 0:200:2], Act.Copy,
                                     bias=w["b2"], scale=w2[1])

            def tail_batch(lo, hi, c0=0, cn=C):
                """S5..S9 batched over supertiles [lo, hi) x groups
                [c0, c0+cn) (fp32)."""
                sl = slice(lo, hi)
                cs = slice(c0, c0 + cn)
                R3 = r3v[:, sl, cs]
                Y4 = y4v[:, sl, cs]
                nc.scalar.activation(Y4, R3[:, :, :, 0:50:2], Act.Copy,
                                     bias=w["b4"], scale=w4[1])
                stt(Y4, R3[:, :, :, 1:50:2], w4[2], Y4, Alu.mult, Alu.add)
                stt(Y4[:, :, :, 1:25], R3[:, :, :, 1:48:2], w4[0],
                    Y4[:, :, :, 1:25], Alu.mult, Alu.add)
                stt(Y4[:, :, :, 0:24], R3[:, :, :, 2:49:2], w4[3],
                    Y4[:, :, :, 0:24], Alu.mult, Alu.add)
                R5 = r5v[:, sl, cs]
                stt(R5, Y4[:, :, :, 0:24:2], 0.0, Y4[:, :, :, 1:25:2],
                    Alu.max, Alu.max)
                Y6 = y6v[:, sl, cs]
                nc.scalar.activation(Y6, R5[:, :, :, 0:12:2], Act.Copy,
                                     bias=w["b6"], scale=w6[1])
                stt(Y6, R5[:, :, :, 1:12:2], w6[2], Y6, Alu.mult, Alu.add)
                stt(Y6[:, :, :, 1:6], R5[:, :, :, 1:10:2], w6[0],
                    Y6[:, :, :, 1:6], Alu.mult, Alu.add)
                stt(Y6[:, :, :, 0:5], R5[:, :, :, 2:11:2], w6[3],
                    Y6[:, :, :, 0:5], Alu.mult, Alu.add)
                R7 = r7v[:, sl, cs]
                stt(R7, Y6[:, :, :, 0:6:2], 0.0, Y6[:, :, :, 1:6:2],
                    Alu.max, Alu.max)
                FB = fbv[:, sl, cs]
                nc.scalar.activation(FB, R7[:, :, :, 0], Act.Copy,
                                     bias=w["b8"], scale=w8[0])
                stt(FB, R7[:, :, :, 1], w8[1], FB, Alu.mult, Alu.add)
                stt(fmv[:, sl, cs], R7[:, :, :, 2], w8[2], FB, Alu.mult, Alu.add)

            for s in range(S_PER_CORE):
                st[s] = {"x": []}
                for q in range(4):
                    xh = xpool.tile([128, 2 * T], F32)
                    nc.sync.dma_start(
                        xh[:].rearrange("p (c t) -> p c t", c=2),
                        x4[s][:, q * 2:(q + 1) * 2])
                    st[s]["x"].append(xh)

                if s >= 2:
                    vec_taps_s4(s - 2)
                    st.pop(s - 2)
                if s == 5:
                    tail_batch(0, 4)
                elif s == 7:
                    tail_batch(4, 6)
                if s >= 1:
                    vec_add_pools(s - 1)
                sca_ae_ao(s)
                if s >= 1:
                    sca_s3base(s - 1)

            # drain: taps/S4 of s=6 + its tail chew on DVE while the last
            # DMAs stream; then supertile 7 runs at QUARTER granularity with
            # a 1-quarter deferral so only ~3us of compute chains after the
            # last DMA byte (instead of a full-supertile pipeline).
            r1_7 = wk.tile([128, C * 200], F16)
            y2_7 = wk.tile([128, C * 100], F16)
            st[7]["r1"] = r1_7
            st[7]["y2"] = y2_7
            vec_taps_s4(6)
            tail_batch(6, 7)
            vec_add_q(7, 0)
            vec_pools_q(7, 0)
            sca_s3base_q(7, 0)
            vec_add_q(7, 1)
            vec_pools_q(7, 1)
            sca_s3base_q(7, 1)
            vec_taps_s4(7, 0, 2)
            vec_add_q(7, 2)
            vec_pools_q(7, 2)
            sca_s3base_q(7, 2)
            vec_taps_s4(7, 2, 2)
            tail_batch(7, 8, 0, 4)
            vec_add_q(7, 3)
            vec_pools_q(7, 3)
            sca_s3base_q(7, 3)
            vec_taps_s4(7, 4, 2)
            vec_taps_s4(7, 6, 2)
            tail_batch(7, 8, 4, 4)

            # classifier weights load late so the first x DMA issues first
            nc.sync.dma_start(clsw[:], clswt[:])
            # classifier: logits[s, cls] = sum_c featmat[:, c::8].T @ clsw
            lg = psum.tile([8, 3], F32)
            for c in range(C):
                nc.tensor.matmul(lg[:], featmat[:, c::8], clsw[:, c * 3:(c + 1) * 3],
                                 start=(c == 0), stop=(c == C - 1))
            if any(v != 0.0 for v in w["cls_b"]):
                lgs = const.tile([8, 3], F32)
                nc.vector.tensor_copy(lgs[:], lg[:])
                for cls in range(3):
                    if w["cls_b"][cls] != 0.0:
                        nc.vector.tensor_scalar_add(lgs[:, cls:cls + 1],
                                                    lgs[:, cls:cls + 1],
                                                    w["cls_b"][cls])
                lsrc = lgs[:]
            else:
                lsrc = lg[:]   # zero bias: reduce + Exp read PSUM directly
            # softmax (max-subtracted, like jax.nn.softmax)
            nmx = const.tile([8, 1], F32)
            nc.vector.tensor_reduce(nmx[:], lsrc, mybir.AxisListType.X, Alu.max,
                                    negate=True)
            ex = const.tile([8, 3], F32)
            smv = const.tile([8, 1], F32)
            nc.scalar.activation(ex[:], lsrc, Act.Exp, bias=nmx[:], scale=1.0,
                                 accum_out=smv[:])
            ri = const.tile([8, 1], F32)
            nc.vector.reciprocal(ri[:], smv[:])
            pr = const.tile([8, 3], F32)
            nc.vector.tensor_scalar(pr[:], ex[:], ri[:], None, Alu.mult)
            nc.sync.dma_start(out[:], pr[:])

    nc.compile()
    return nc


def _extract_weights(inputs):
    f = lambda a: [float(v) for v in np.asarray(a).reshape(-1)]
    return dict(
        w00=f(inputs["c0_w"])[0], w01=f(inputs["c0_w"])[1], b0=f(inputs["c0_b"])[0],
        w2=f(inputs["c2_w"]), b2=f(inputs["c2_b"])[0],
        w4=f(inputs["c4_w"]), b4=f(inputs["c4_b"])[0],
        w6=f(inputs["c6_w"]), b6=f(inputs["c6_b"])[0],
        w8=f(inputs["c8_w"]), b8=f(inputs["c8_b"])[0],
        cls_b=f(inputs["cls_b"]),
    )


def _run(inputs, trace=False, trace_kwargs=None):
    w = _extract_weights(inputs)
    key = tuple(np.asarray(
        [w["w00"], w["w01"], w["b0"]] + w["w2"] + [w["b2"]] + w["w4"] + [w["b4"]]
        + w["w6"] + [w["b6"]] + w["w8"] + [w["b8"]] + w["cls_b"], np.float64
    ).tobytes())
    if key not in _CACHE:
        _CACHE[key] = _build(w)
    nc = _CACHE[key]

    x = np.ascontiguousarray(np.asarray(inputs["x"], dtype=np.float32))
    xf = x.reshape(BS * NN, T)
    cls_w = np.asarray(inputs["cls_w"], dtype=np.float32)       # (3, 1024)
    clsT = np.empty((128, 24), np.float32)
    for c in range(C):
        clsT[:, c * 3:(c + 1) * 3] = cls_w[:, c * 128:(c + 1) * 128].T

    rows_per_core = BS * NN // N_CORES
    in_maps = [
        {"x": np.ascontiguousarray(xf[i * rows_per_core:(i + 1) * rows_per_core]),
         "clswt": clsT}
        for i in range(N_CORES)
    ]
    res = run_bass_kernel_spmd(nc, in_maps, list(range(N_CORES)), trace=trace,
                               **(trace_kwargs or {}))
    out = np.concatenate([np.asarray(res.results[i]["out"]) for i in range(N_CORES)],
                         axis=0).astype(np.float32)
    return out, res


def kernel(**inputs):
    out, _ = _run(inputs, trace=False)
    return out


# revision 17
# speedup vs baseline: 1.0943x; 1.0943x over previous
"""Trainium2 Bass kernel for nn_DeepTimeGraphNet (per-row conv/pool pyramid + classifier).

Contract: kernel(**inputs) takes the FULL unsharded inputs (keys as in
setup_inputs()) and returns the FULL (64, 3) softmax output.

Sharding: pure data parallel over batch. Core i handles batch rows
[8i, 8i+8) = 8192 (batch, node) rows of length 1200, processed as 8
supertiles of 1024 rows = 128 SBUF partitions x 8 column groups.

v3 design, from measured engine rates (DVE stt/TT and ScalarE ACT are all
~0.96 el/ns on fp32 regardless of stride/dtype, EXCEPT contiguous fp16
tensor_tensor which hits ~1.98 el/ns; GpSimd cannot run tensor ops):

- S1 (conv0 k2 s2): ScalarE computes both taps as separate fp16 buffers
  (ae = w00*x_even + b0, ao = w01*x_odd) via ACTIVATE at ~1 el/ns, and
  DVE combines them with one contiguous fp16 TT add at ~2 el/ns. This
  moves ~half of conv0 off the (otherwise oversubscribed) DVE.
- S2 (maxpool3+relu): fp16 TT max + fp16 stt max-max (relu fused via the
  scalar-0 slot), stride-3 reads.
- S3 (conv2 k4 s2 p1): ScalarE base tap (bias), 3 DVE fp16 stt taps.
- S4 (maxpool2+relu): one stt, fp16 srcs -> fp32 r3 staging.
- 2-deep software pipeline: iteration s emits DVE taps/S4 of s-2 and
  DVE add/pools of s-1 while ScalarE loads supertile s, so neither
  engine ever waits on the other's round trip.
- Tail (conv4..conv8, 50->1, fp32) batched over supertile groups
  [0,4) [4,6) [6,8) placed in DMA windows; PE matmul classifier +
  exact softmax.

Per-supertile engine budget: DVE ~11.5us, ScalarE ~11.4us, both under
the 12.5us DMA window (39.3 MB/core over 16 DMA engines x ~24.6 B/ns
~= 100us floor), so the stream is DMA-paced.
"""
import os
import sys

for _p in ("/root/.axon_site/_ro/trn_rl_repo", "/opt/trn_rl_repo"):
    if os.path.isdir(_p) and _p not in sys.path:
        sys.path.insert(0, _p)

import numpy as np  # noqa: E402

import concourse.bacc as bacc  # noqa: E402
import concourse.tile as tile  # noqa: E402
from concourse import mybir  # noqa: E402
from concourse.bass_utils import run_bass_kernel_spmd  # noqa: E402

F32 = mybir.dt.float32
F16 = mybir.dt.float16
Alu = mybir.AluOpType
Act = mybir.ActivationFunctionType

BS, NN, T = 64, 1024, 1200
N_CORES = 8
S_PER_CORE = 8          # supertiles per core; each = 1024 rows (one batch row)
C = 8                   # column groups per supertile (128 rows each)

_CACHE = {}


def _build(w):
    """Build + compile the per-core SPMD program with weights baked in."""
    nc = bacc.Bacc("TRN2", target_bir_lowering=False, debug=False)
    x = nc.dram_tensor("x", [S_PER_CORE * C * 128, T], F32, kind="ExternalInput")
    clswt = nc.dram_tensor("clswt", [128, 24], F32, kind="ExternalInput")
    out = nc.dram_tensor("out", [8, 3], F32, kind="ExternalOutput")

    w2, w4, w6, w8 = w["w2"], w["w4"], w["w6"], w["w8"]
    stt = nc.vector.scalar_tensor_tensor

    with tile.TileContext(nc) as tc:
        with (
            tc.tile_pool(name="xpool", bufs=8) as xpool,
            tc.tile_pool(name="aeo", bufs=16) as aeo,
            tc.tile_pool(name="wk", bufs=2) as wk,
            tc.tile_pool(name="const", bufs=1) as const,
            tc.tile_pool(name="psum", bufs=1, space="PSUM") as psum,
        ):
            clsw = const.tile([128, 24], F32)
            featmat = const.tile([128, 64], F32)

            y0 = const.tile([128, C * 600], F16)
            y0v = y0[:].rearrange("p (c t) -> p c t", c=C)
            t01 = const.tile([128, C * 200], F16)
            t01v = t01[:].rearrange("p (c t) -> p c t", c=C)

            # persistent staging for the batched small stages (fp32)
            r3all = const.tile([128, S_PER_CORE * C * 50], F32)
            y4all = const.tile([128, S_PER_CORE * C * 25], F32)
            r5all = const.tile([128, S_PER_CORE * C * 12], F32)
            y6all = const.tile([128, S_PER_CORE * C * 6], F32)
            r7all = const.tile([128, S_PER_CORE * C * 3], F32)
            fball = const.tile([128, S_PER_CORE * C], F32)
            r3v = r3all[:].rearrange("p (s c t) -> p s c t", s=S_PER_CORE, c=C)
            y4v = y4all[:].rearrange("p (s c t) -> p s c t", s=S_PER_CORE, c=C)
            r5v = r5all[:].rearrange("p (s c t) -> p s c t", s=S_PER_CORE, c=C)
            y6v = y6all[:].rearrange("p (s c t) -> p s c t", s=S_PER_CORE, c=C)
            r7v = r7all[:].rearrange("p (s c t) -> p s c t", s=S_PER_CORE, c=C)
            fbv = fball[:].rearrange("p (s c) -> p s c", s=S_PER_CORE)
            fmv = featmat[:].rearrange("p (s c) -> p s c", s=S_PER_CORE)

            x4 = x[:].rearrange("(s c p) t -> s p c t", s=S_PER_CORE, c=C, p=128)

            st = {}   # per-supertile handles

            def sca_ae_ao(s):
                """S1 taps on ScalarE: ae = w00*x_even + b0, ao = w01*x_odd
                (fp16), per quarter-supertile (fine slot recycling keeps the
                DMA queues fed and completions arriving every ~3us)."""
                d = st[s]
                d["ae"], d["ao"] = [], []
                for q in range(4):
                    xb = d["x"][q][:]
                    ae = aeo.tile([128, 1200], F16)
                    ao = aeo.tile([128, 1200], F16)
                    nc.scalar.activation(ae[:], xb[:, 0:2400:2], Act.Copy,
                                         bias=w["b0"], scale=w["w00"])
                    nc.scalar.activation(ao[:], xb[:, 1:2400:2], Act.Copy,
                                         bias=0.0, scale=w["w01"])
                    d["ae"].append(ae)
                    d["ao"].append(ao)

            def vec_add_pools(s):
                """DVE: y0 = ae + ao (contiguous fp16 TT, 2x rate), then
                maxpool3 + fused relu -> r1 (fp16)."""
                d = st[s]
                for q in range(4):
                    nc.vector.tensor_tensor(y0[:, q * 1200:(q + 1) * 1200],
                                            d["ae"][q][:], d["ao"][q][:],
                                            Alu.add)
                nc.vector.tensor_tensor(t01v, y0v[:, :, 0:600:3],
                                        y0v[:, :, 1:600:3], Alu.max)
                r1 = wk.tile([128, C * 200], F16)
                d["r1"] = r1
                stt(r1[:].rearrange("p (c t) -> p c t", c=C), t01v, 0.0,
                    y0v[:, :, 2:600:3], Alu.max, Alu.max)

            def sca_s3base(s):
                """ScalarE base tap of conv2: y2 = w2[1]*r1_even + b2."""
                d = st[s]
                y2 = wk.tile([128, C * 100], F16)
                d["y2"] = y2
                r1v = d["r1"][:].rearrange("p (c t) -> p c t", c=C)
                nc.scalar.activation(y2[:].rearrange("p (c t) -> p c t", c=C),
                                     r1v[:, :, 0:200:2], Act.Copy,
                                     bias=w["b2"], scale=w2[1])

            def vec_taps_s4(s, c0=0, cn=C):
                """DVE taps of conv2 + S4 maxpool2+relu -> fp32 r3."""
                d = st[s]
                cs = slice(c0, c0 + cn)
                r1v = d["r1"][:].rearrange("p (c t) -> p c t", c=C)[:, cs]
                y2v = d["y2"][:].rearrange("p (c t) -> p c t", c=C)[:, cs]
                stt(y2v, r1v[:, :, 1:200:2], w2[2], y2v, Alu.mult, Alu.add)
                stt(y2v[:, :, 1:100], r1v[:, :, 1:198:2], w2[0],
                    y2v[:, :, 1:100], Alu.mult, Alu.add)
                stt(y2v[:, :, 0:99], r1v[:, :, 2:199:2], w2[3],
                    y2v[:, :, 0:99], Alu.mult, Alu.add)
                stt(r3v[:, s, cs], y2v[:, :, 0:100:2], 0.0, y2v[:, :, 1:100:2],
                    Alu.max, Alu.max)

            def vec_add_q(s, q):
                d = st[s]
                nc.vector.tensor_tensor(y0[:, q * 1200:(q + 1) * 1200],
                                        d["ae"][q][:], d["ao"][q][:], Alu.add)

            def vec_pools_q(s, q):
                d = st[s]
                cs = slice(2 * q, 2 * q + 2)
                nc.vector.tensor_tensor(t01v[:, cs], y0v[:, cs, 0:600:3],
                                        y0v[:, cs, 1:600:3], Alu.max)
                stt(d["r1"][:].rearrange("p (c t) -> p c t", c=C)[:, cs],
                    t01v[:, cs], 0.0, y0v[:, cs, 2:600:3], Alu.max, Alu.max)

            def sca_s3base_q(s, q):
                d = st[s]
                cs = slice(2 * q, 2 * q + 2)
                r1v = d["r1"][:].rearrange("p (c t) -> p c t", c=C)
                y2v = d["y2"][:].rearrange("p (c t) -> p c t", c=C)
                nc.scalar.activation(y2v[:, cs], r1v[:, cs, 0:200:2], Act.Copy,
                                     bias=w["b2"], scale=w2[1])

            def tail_batch(lo, hi, c0=0, cn=C):
                """S5..S9 batched over supertiles [lo, hi) x groups
                [c0, c0+cn) (fp32)."""
                sl = slice(lo, hi)
                cs = slice(c0, c0 + cn)
                R3 = r3v[:, sl, cs]
                Y4 = y4v[:, sl, cs]
                nc.scalar.activation(Y4, R3[:, :, :, 0:50:2], Act.Copy,
                                     bias=w["b4"], scale=w4[1])
                stt(Y4, R3[:, :, :, 1:50:2], w4[2], Y4, Alu.mult, Alu.add)
                stt(Y4[:, :, :, 1:25], R3[:, :, :, 1:48:2], w4[0],
                    Y4[:, :, :, 1:25], Alu.mult, Alu.add)
                stt(Y4[:, :, :, 0:24], R3[:, :, :, 2:49:2], w4[3],
                    Y4[:, :, :, 0:24], Alu.mult, Alu.add)
                R5 = r5v[:, sl, cs]
                stt(R5, Y4[:, :, :, 0:24:2], 0.0, Y4[:, :, :, 1:25:2],
                    Alu.max, Alu.max)
                Y6 = y6v[:, sl, cs]
                nc.scalar.activation(Y6, R5[:, :, :, 0:12:2], Act.Copy,
                                     bias=w["b6"], scale=w6[1])
                stt(Y6, R5[:, :, :, 1:12:2], w6[2], Y6, Alu.mult, Alu.add)
                stt(Y6[:, :, :, 1:6], R5[:, :, :, 1:10:2], w6[0],
                    Y6[:, :, :, 1:6], Alu.mult, Alu.add)
                stt(Y6[:, :, :, 0:5], R5[:, :, :, 2:11:2], w6[3],
                    Y6[:, :, :, 0:5], Alu.mult, Alu.add)
                R7 = r7v[:, sl, cs]
                stt(R7, Y6[:, :, :, 0:6:2], 0.0, Y6[:, :, :, 1:6:2],
                    Alu.max, Alu.max)
                FB = fbv[:, sl, cs]
                nc.scalar.activation(FB, R7[:, :, :, 0], Act.Copy,
                                     bias=w["b8"], scale=w8[0])
                stt(FB, R7[:, :, :, 1], w8[1], FB, Alu.mult, Alu.add)
                stt(fmv[:, sl, cs], R7[:, :, :, 2], w8[2], FB, Alu.mult, Alu.add)

            for s in range(S_PER_CORE):
                st[s] = {"x": []}
                for q in range(4):
                    xh = xpool.tile([128, 2 * T], F32)
                    nc.sync.dma_start(
                        xh[:].rearrange("p (c t) -> p c t", c=2),
                        x4[s][:, q * 2:(q + 1) * 2])
                    st[s]["x"].append(xh)

                if s >= 2:
                    vec_taps_s4(s - 2)
                    st.pop(s - 2)
                if s == 5:
                    tail_batch(0, 4)
                elif s == 7:
                    tail_batch(4, 6)
                if s >= 1:
                    vec_add_pools(s - 1)
                sca_ae_ao(s)
                if s >= 1:
                    sca_s3base(s - 1)

            # drain: taps/S4 of s=6 + its tail chew on DVE while the last
            # DMAs stream; then supertile 7 runs at QUARTER granularity with
            # a 1-quarter deferral so only ~3us of compute chains after the
            # last DMA byte (instead of a full-supertile pipeline).
            r1_7 = wk.tile([128, C * 200], F16)
            y2_7 = wk.tile([128, C * 100], F16)
            st[7]["r1"] = r1_7
            st[7]["y2"] = y2_7
            vec_taps_s4(6)
            tail_batch(6, 7)
            vec_add_q(7, 0)
            vec_pools_q(7, 0)
            sca_s3base_q(7, 0)
            vec_add_q(7, 1)
            vec_pools_q(7, 1)
            sca_s3base_q(7, 1)
            vec_taps_s4(7, 0, 2)
            vec_add_q(7, 2)
            vec_pools_q(7, 2)
            sca_s3base_q(7, 2)
            vec_taps_s4(7, 2, 2)
            tail_batch(7, 8, 0, 4)
            vec_add_q(7, 3)
            vec_pools_q(7, 3)
            sca_s3base_q(7, 3)
            vec_taps_s4(7, 4, 2)
            vec_taps_s4(7, 6, 2)
            tail_batch(7, 8, 4, 4)

            # classifier weights load late so the first x DMA issues first
            nc.sync.dma_start(clsw[:], clswt[:])
            # classifier: logits[s, cls] = sum_c featmat[:, c::8].T @ clsw
            lg = psum.tile([8, 3], F32)
            for c in range(C):
                nc.tensor.matmul(lg[:], featmat[:, c::8], clsw[:, c * 3:(c + 1) * 3],
                                 start=(c == 0), stop=(c == C - 1))
            if any(v != 0.0 for v in w["cls_b"]):
                lgs = const.tile([8, 3], F32)
                nc.vector.tensor_copy(lgs[:], lg[:])
                for cls in range(3):
                    if w["cls_b"][cls] != 0.0:
                        nc.vector.tensor_scalar_add(lgs[:, cls:cls + 1],
                                                    lgs[:, cls:cls + 1],
                                                    w["cls_b"][cls])
                lsrc = lgs[:]
            else:
                lsrc = lg[:]   # zero bias: reduce + Exp read PSUM directly
            # softmax (max-subtracted, like jax.nn.softmax)
            nmx = const.tile([8, 1], F32)
            nc.vector.tensor_reduce(nmx[:], lsrc, mybir.AxisListType.X, Alu.max,
                                    negate=True)
            ex = const.tile([8, 3], F32)
            smv = const.tile([8, 1], F32)
            nc.scalar.activation(ex[:], lsrc, Act.Exp, bias=nmx[:], scale=1.0,
                                 accum_out=smv[:])
            ri = const.tile([8, 1], F32)
            nc.vector.reciprocal(ri[:], smv[:])
            pr = const.tile([8, 3], F32)
            nc.vector.tensor_scalar(pr[:], ex[:], ri[:], None, Alu.mult)
            nc.sync.dma_start(out[:], pr[:])

    nc.compile()
    return nc


def _extract_weights(inputs):
    f = lambda a: [float(v) for v in np.asarray(a).reshape(-1)]
    return dict(
        w00=f(inputs["c0_w"])[0], w01=f(inputs["c0_w"])[1], b0=f(inputs["c0_b"])[0],
        w2=f(inputs["c2_w"]), b2=f(inputs["c2_b"])[0],
        w4=f(inputs["c4_w"]), b4=f(inputs["c4_b"])[0],
        w6=f(inputs["c6_w"]), b6=f(inputs["c6_b"])[0],
        w8=f(inputs["c8_w"]), b8=f(inputs["c8_b"])[0],
        cls_b=f(inputs["cls_b"]),
    )


def _run(inputs, trace=False, trace_kwargs=None):
    w = _extract_weights(inputs)
    key = tuple(np.asarray(
        [w["w00"], w["w01"], w["b0"]] + w["w2"] + [w["b2"]] + w["w4"] + [w["b4"]]
        + w["w6"] + [w["b6"]] + w["w8"] + [w["b8"]] + w["cls_b"], np.float64
    ).tobytes())
    if key not in _CACHE:
        _CACHE[key] = _build(w)
    nc = _CACHE[key]

    x = np.ascontiguousarray(np.asarray(inputs["x"], dtype=np.float32))
    xf = x.reshape(BS * NN, T)
    cls_w = np.asarray(inputs["cls_w"], dtype=np.float32)       # (3, 1024)
    clsT = np.empty((128, 24), np.float32)
    for c in range(C):
        clsT[:, c * 3:(c + 1) * 3] = cls_w[:, c * 128:(c + 1) * 128].T

    rows_per_core = BS * NN // N_CORES
    in_maps = [
        {"x": np.ascontiguousarray(xf[i * rows_per_core:(i + 1) * rows_per_core]),
         "clswt": clsT}
        for i in range(N_CORES)
    ]
    res = run_bass_kernel_spmd(nc, in_maps, list(range(N_CORES)), trace=trace,
                               **(trace_kwargs or {}))
    out = np.concatenate([np.asarray(res.results[i]["out"]) for i in range(N_CORES)],
                         axis=0).astype(np.float32)
    return out, res


def kernel(**inputs):
    out, _ = _run(inputs, trace=False)
    return out


# revision 20
# speedup vs baseline: 1.1186x; 1.0221x over previous
"""Trainium2 Bass kernel for nn_DeepTimeGraphNet (per-row conv/pool pyramid + classifier).

Contract: kernel(**inputs) takes the FULL unsharded inputs (keys as in
setup_inputs()) and returns the FULL (64, 3) softmax output.

Sharding: pure data parallel over batch. Core i handles batch rows
[8i, 8i+8) = 8192 (batch, node) rows of length 1200, processed as 8
supertiles of 1024 rows = 128 SBUF partitions x 8 column groups.

v3 design, from measured engine rates (DVE stt/TT and ScalarE ACT are all
~0.96 el/ns on fp32 regardless of stride/dtype, EXCEPT contiguous fp16
tensor_tensor which hits ~1.98 el/ns; GpSimd cannot run tensor ops):

- S1 (conv0 k2 s2): ScalarE computes both taps as separate fp16 buffers
  (ae = w00*x_even + b0, ao = w01*x_odd) via ACTIVATE at ~1 el/ns, and
  DVE combines them with one contiguous fp16 TT add at ~2 el/ns. This
  moves ~half of conv0 off the (otherwise oversubscribed) DVE.
- S2 (maxpool3+relu): fp16 TT max + fp16 stt max-max (relu fused via the
  scalar-0 slot), stride-3 reads.
- S3 (conv2 k4 s2 p1): ScalarE base tap (bias), 3 DVE fp16 stt taps.
- S4 (maxpool2+relu): one stt, fp16 srcs -> fp32 r3 staging.
- 2-deep software pipeline: iteration s emits DVE taps/S4 of s-2 and
  DVE add/pools of s-1 while ScalarE loads supertile s, so neither
  engine ever waits on the other's round trip.
- Tail (conv4..conv8, 50->1, fp32) batched over supertile groups
  [0,4) [4,6) [6,8) placed in DMA windows; PE matmul classifier +
  exact softmax.

Per-supertile engine budget: DVE ~11.5us, ScalarE ~11.4us, both under
the 12.5us DMA window (39.3 MB/core over 16 DMA engines x ~24.6 B/ns
~= 100us floor), so the stream is DMA-paced.
"""
import os
import sys

for _p in ("/root/.axon_site/_ro/trn_rl_repo", "/opt/trn_rl_repo"):
    if os.path.isdir(_p) and _p not in sys.path:
        sys.path.insert(0, _p)

import numpy as np  # noqa: E402

import concourse.bacc as bacc  # noqa: E402
import concourse.tile as tile  # noqa: E402
from concourse import mybir  # noqa: E402
from concourse.bass_utils import run_bass_kernel_spmd  # noqa: E402

F32 = mybir.dt.float32
F16 = mybir.dt.float16
Alu = mybir.AluOpType
Act = mybir.ActivationFunctionType

BS, NN, T = 64, 1024, 1200
N_CORES = 8
S_PER_CORE = 8          # supertiles per core; each = 1024 rows (one batch row)
C = 8                   # column groups per supertile (128 rows each)

_CACHE = {}


def _build(w):
    """Build + compile the per-core SPMD program with weights baked in."""
    nc = bacc.Bacc("TRN2", target_bir_lowering=False, debug=False)
    x = nc.dram_tensor("x", [S_PER_CORE * C * 128, T], F32, kind="ExternalInput")
    clswt = nc.dram_tensor("clswt", [128, 200], F32, kind="ExternalInput")
    out = nc.dram_tensor("out", [8, 3], F32, kind="ExternalOutput")

    w2, w4, w6, w8 = w["w2"], w["w4"], w["w6"], w["w8"]
    stt = nc.vector.scalar_tensor_tensor

    with tile.TileContext(nc) as tc:
        with (
            tc.tile_pool(name="xpool", bufs=8) as xpool,
            tc.tile_pool(name="aeo", bufs=16) as aeo,
            tc.tile_pool(name="wk", bufs=2) as wk,
            tc.tile_pool(name="const", bufs=1) as const,
            tc.tile_pool(name="psum", bufs=1, space="PSUM") as psum,
        ):
            clsw = const.tile([128, 200], F32)
            featmat = const.tile([128, 64], F32)

            y0 = const.tile([128, C * 600], F16)
            y0v = y0[:].rearrange("p (c t) -> p c t", c=C)
            t01 = const.tile([128, C * 200], F16)
            t01v = t01[:].rearrange("p (c t) -> p c t", c=C)

            # persistent staging for the batched small stages (fp32)
            r3all = const.tile([128, S_PER_CORE * C * 50], F32)
            y4all = const.tile([128, S_PER_CORE * C * 25], F32)
            r5all = const.tile([128, S_PER_CORE * C * 12], F32)
            y6all = const.tile([128, S_PER_CORE * C * 6], F32)
            r7all = const.tile([128, S_PER_CORE * C * 3], F32)
            fball = const.tile([128, S_PER_CORE * C], F32)
            r3v = r3all[:].rearrange("p (s c t) -> p s c t", s=S_PER_CORE, c=C)
            y4v = y4all[:].rearrange("p (s c t) -> p s c t", s=S_PER_CORE, c=C)
            r5v = r5all[:].rearrange("p (s c t) -> p s c t", s=S_PER_CORE, c=C)
            y6v = y6all[:].rearrange("p (s c t) -> p s c t", s=S_PER_CORE, c=C)
            r7v = r7all[:].rearrange("p (s c t) -> p s c t", s=S_PER_CORE, c=C)
            fbv = fball[:].rearrange("p (s c) -> p s c", s=S_PER_CORE)
            fmv = featmat[:].rearrange("p (s c) -> p s c", s=S_PER_CORE)

            # partition p holds 64 CONSECUTIVE dram rows (r = 64p + 8s + c):
            # each quarter-DMA is then 128 descriptors of 9600B contiguous
            # (2 rows) instead of 256x4800B, halving the descriptor-fetch
            # traffic on the BD-ring host engine (the stream straggler).
            x4 = x[:].rearrange("(p s c) t -> s p c t", p=128, s=S_PER_CORE, c=C)

            st = {}   # per-supertile handles

            def sca_ae_ao(s):
                """S1 taps on ScalarE: ae = w00*x_even + b0, ao = w01*x_odd
                (fp16), per quarter-supertile (fine slot recycling keeps the
                DMA queues fed and completions arriving every ~3us)."""
                d = st[s]
                d["ae"], d["ao"] = [], []
                for q in range(4):
                    xb = d["x"][q][:]
                    ae = aeo.tile([128, 1200], F16)
                    ao = aeo.tile([128, 1200], F16)
                    nc.scalar.activation(ae[:], xb[:, 0:2400:2], Act.Copy,
                                         bias=w["b0"], scale=w["w00"])
                    nc.scalar.activation(ao[:], xb[:, 1:2400:2], Act.Copy,
                                         bias=0.0, scale=w["w01"])
                    d["ae"].append(ae)
                    d["ao"].append(ao)

            def vec_add_pools(s):
                """DVE: y0 = ae + ao (contiguous fp16 TT, 2x rate), then
                maxpool3 + fused relu -> r1 (fp16)."""
                d = st[s]
                for q in range(4):
                    nc.vector.tensor_tensor(y0[:, q * 1200:(q + 1) * 1200],
                                            d["ae"][q][:], d["ao"][q][:],
                                            Alu.add)
                nc.vector.tensor_tensor(t01v, y0v[:, :, 0:600:3],
                                        y0v[:, :, 1:600:3], Alu.max)
                r1 = wk.tile([128, C * 200], F16)
                d["r1"] = r1
                stt(r1[:].rearrange("p (c t) -> p c t", c=C), t01v, 0.0,
                    y0v[:, :, 2:600:3], Alu.max, Alu.max)

            def sca_s3base(s):
                """ScalarE base tap of conv2: y2 = w2[1]*r1_even + b2."""
                d = st[s]
                y2 = wk.tile([128, C * 100], F16)
                d["y2"] = y2
                r1v = d["r1"][:].rearrange("p (c t) -> p c t", c=C)
                nc.scalar.activation(y2[:].rearrange("p (c t) -> p c t", c=C),
                                     r1v[:, :, 0:200:2], Act.Copy,
                                     bias=w["b2"], scale=w2[1])

            def vec_taps_s4(s, c0=0, cn=C):
                """DVE taps of conv2 + S4 maxpool2+relu -> fp32 r3."""
                d = st[s]
                cs = slice(c0, c0 + cn)
                r1v = d["r1"][:].rearrange("p (c t) -> p c t", c=C)[:, cs]
                y2v = d["y2"][:].rearrange("p (c t) -> p c t", c=C)[:, cs]
                stt(y2v, r1v[:, :, 1:200:2], w2[2], y2v, Alu.mult, Alu.add)
                stt(y2v[:, :, 1:100], r1v[:, :, 1:198:2], w2[0],
                    y2v[:, :, 1:100], Alu.mult, Alu.add)
                stt(y2v[:, :, 0:99], r1v[:, :, 2:199:2], w2[3],
                    y2v[:, :, 0:99], Alu.mult, Alu.add)
                stt(r3v[:, s, cs], y2v[:, :, 0:100:2], 0.0, y2v[:, :, 1:100:2],
                    Alu.max, Alu.max)

            def vec_add_q(s, q):
                d = st[s]
                nc.vector.tensor_tensor(y0[:, q * 1200:(q + 1) * 1200],
                                        d["ae"][q][:], d["ao"][q][:], Alu.add)

            def vec_pools_q(s, q):
                d = st[s]
                cs = slice(2 * q, 2 * q + 2)
                nc.vector.tensor_tensor(t01v[:, cs], y0v[:, cs, 0:600:3],
                                        y0v[:, cs, 1:600:3], Alu.max)
                stt(d["r1"][:].rearrange("p (c t) -> p c t", c=C)[:, cs],
                    t01v[:, cs], 0.0, y0v[:, cs, 2:600:3], Alu.max, Alu.max)

            def sca_s3base_q(s, q):
                d = st[s]
                cs = slice(2 * q, 2 * q + 2)
                r1v = d["r1"][:].rearrange("p (c t) -> p c t", c=C)
                y2v = d["y2"][:].rearrange("p (c t) -> p c t", c=C)
                nc.scalar.activation(y2v[:, cs], r1v[:, cs, 0:200:2], Act.Copy,
                                     bias=w["b2"], scale=w2[1])

            def tail_batch(lo, hi, c0=0, cn=C):
                """S5..S9 batched over supertiles [lo, hi) x groups
                [c0, c0+cn) (fp32)."""
                sl = slice(lo, hi)
                cs = slice(c0, c0 + cn)
                R3 = r3v[:, sl, cs]
                Y4 = y4v[:, sl, cs]
                nc.scalar.activation(Y4, R3[:, :, :, 0:50:2], Act.Copy,
                                     bias=w["b4"], scale=w4[1])
                stt(Y4, R3[:, :, :, 1:50:2], w4[2], Y4, Alu.mult, Alu.add)
                stt(Y4[:, :, :, 1:25], R3[:, :, :, 1:48:2], w4[0],
                    Y4[:, :, :, 1:25], Alu.mult, Alu.add)
                stt(Y4[:, :, :, 0:24], R3[:, :, :, 2:49:2], w4[3],
                    Y4[:, :, :, 0:24], Alu.mult, Alu.add)
                R5 = r5v[:, sl, cs]
                stt(R5, Y4[:, :, :, 0:24:2], 0.0, Y4[:, :, :, 1:25:2],
                    Alu.max, Alu.max)
                Y6 = y6v[:, sl, cs]
                nc.scalar.activation(Y6, R5[:, :, :, 0:12:2], Act.Copy,
                                     bias=w["b6"], scale=w6[1])
                stt(Y6, R5[:, :, :, 1:12:2], w6[2], Y6, Alu.mult, Alu.add)
                stt(Y6[:, :, :, 1:6], R5[:, :, :, 1:10:2], w6[0],
                    Y6[:, :, :, 1:6], Alu.mult, Alu.add)
                stt(Y6[:, :, :, 0:5], R5[:, :, :, 2:11:2], w6[3],
                    Y6[:, :, :, 0:5], Alu.mult, Alu.add)
                R7 = r7v[:, sl, cs]
                stt(R7, Y6[:, :, :, 0:6:2], 0.0, Y6[:, :, :, 1:6:2],
                    Alu.max, Alu.max)
                FB = fbv[:, sl, cs]
                nc.scalar.activation(FB, R7[:, :, :, 0], Act.Copy,
                                     bias=w["b8"], scale=w8[0])
                stt(FB, R7[:, :, :, 1], w8[1], FB, Alu.mult, Alu.add)
                stt(fmv[:, sl, cs], R7[:, :, :, 2], w8[2], FB, Alu.mult, Alu.add)

            for s in range(S_PER_CORE):
                st[s] = {"x": []}
                for q in range(4):
                    xh = xpool.tile([128, 2 * T], F32)
                    nc.sync.dma_start(
                        xh[:].rearrange("p (c t) -> p c t", c=2),
                        x4[s][:, q * 2:(q + 1) * 2])
                    st[s]["x"].append(xh)

                if s >= 2:
                    vec_taps_s4(s - 2)
                    st.pop(s - 2)
                if s == 5:
                    tail_batch(0, 4)
                elif s == 7:
                    tail_batch(4, 6)
                if s >= 1:
                    vec_add_pools(s - 1)
                sca_ae_ao(s)
                if s >= 1:
                    sca_s3base(s - 1)

            # drain: taps/S4 of s=6 + its tail chew on DVE while the last
            # DMAs stream; then supertile 7 runs at QUARTER granularity with
            # a 1-quarter deferral so only ~3us of compute chains after the
            # last DMA byte (instead of a full-supertile pipeline).
            r1_7 = wk.tile([128, C * 200], F16)
            y2_7 = wk.tile([128, C * 100], F16)
            st[7]["r1"] = r1_7
            st[7]["y2"] = y2_7
            vec_taps_s4(6)
            tail_batch(6, 7)
            vec_add_q(7, 0)
            vec_pools_q(7, 0)
            sca_s3base_q(7, 0)
            vec_add_q(7, 1)
            vec_pools_q(7, 1)
            sca_s3base_q(7, 1)
            vec_taps_s4(7, 0, 2)
            vec_add_q(7, 2)
            vec_pools_q(7, 2)
            sca_s3base_q(7, 2)
            vec_taps_s4(7, 2, 2)
            tail_batch(7, 8, 0, 4)
            vec_add_q(7, 3)
            vec_pools_q(7, 3)
            sca_s3base_q(7, 3)
            vec_taps_s4(7, 4, 2)
            vec_taps_s4(7, 6, 2)
            tail_batch(7, 8, 4, 4)

            # classifier weights load late so the first x DMA issues first
            nc.sync.dma_start(clsw[:], clswt[:])
            # classifier under the consecutive-row mapping: batch b lives on
            # partition block [16b, 16b+16). partial_j[p] = sum_(s,c)
            # feat[p,s,c] * Wj[p,s,c] (stt with accum_out), then one PE
            # matmul against the 0/1 block mask sums each block.
            dum = const.tile([128, 64], F32)
            partial = const.tile([128, 3], F32)
            for j in range(3):
                stt(dum[:], featmat[:, 0:64], 1.0, clsw[:, j * 64:(j + 1) * 64],
                    Alu.mult, Alu.mult, accum_out=partial[:, j:j + 1])
            lg = psum.tile([8, 3], F32)
            nc.tensor.matmul(lg[:], clsw[:, 192:200], partial[:],
                             start=True, stop=True)
            if any(v != 0.0 for v in w["cls_b"]):
                lgs = const.tile([8, 3], F32)
                nc.vector.tensor_copy(lgs[:], lg[:])
                for cls in range(3):
                    if w["cls_b"][cls] != 0.0:
                        nc.vector.tensor_scalar_add(lgs[:, cls:cls + 1],
                                                    lgs[:, cls:cls + 1],
                                                    w["cls_b"][cls])
                lsrc = lgs[:]
            else:
                lsrc = lg[:]   # zero bias: reduce + Exp read PSUM directly
            # softmax (max-subtracted, like jax.nn.softmax)
            nmx = const.tile([8, 1], F32)
            nc.vector.tensor_reduce(nmx[:], lsrc, mybir.AxisListType.X, Alu.max,
                                    negate=True)
            ex = const.tile([8, 3], F32)
            smv = const.tile([8, 1], F32)
            nc.scalar.activation(ex[:], lsrc, Act.Exp, bias=nmx[:], scale=1.0,
                                 accum_out=smv[:])
            ri = const.tile([8, 1], F32)
            nc.vector.reciprocal(ri[:], smv[:])
            pr = const.tile([8, 3], F32)
            nc.vector.tensor_scalar(pr[:], ex[:], ri[:], None, Alu.mult)
            nc.sync.dma_start(out[:], pr[:])

    nc.compile()
    return nc


def _extract_weights(inputs):
    f = lambda a: [float(v) for v in np.asarray(a).reshape(-1)]
    return dict(
        w00=f(inputs["c0_w"])[0], w01=f(inputs["c0_w"])[1], b0=f(inputs["c0_b"])[0],
        w2=f(inputs["c2_w"]), b2=f(inputs["c2_b"])[0],
        w4=f(inputs["c4_w"]), b4=f(inputs["c4_b"])[0],
        w6=f(inputs["c6_w"]), b6=f(inputs["c6_b"])[0],
        w8=f(inputs["c8_w"]), b8=f(inputs["c8_b"])[0],
        cls_b=f(inputs["cls_b"]),
    )


def _run(inputs, trace=False, trace_kwargs=None):
    w = _extract_weights(inputs)
    key = tuple(np.asarray(
        [w["w00"], w["w01"], w["b0"]] + w["w2"] + [w["b2"]] + w["w4"] + [w["b4"]]
        + w["w6"] + [w["b6"]] + w["w8"] + [w["b8"]] + w["cls_b"], np.float64
    ).tobytes())
    if key not in _CACHE:
        _CACHE[key] = _build(w)
    nc = _CACHE[key]

    x = np.ascontiguousarray(np.asarray(inputs["x"], dtype=np.float32))
    xf = x.reshape(BS * NN, T)
    cls_w = np.asarray(inputs["cls_w"], dtype=np.float32)       # (3, 1024)
    # consecutive-row mapping: feat[p, s, c] is dram row 64p + 8s + c ->
    # node 64*(p%16) + 8s + c, batch block b = p//16.
    clsT = np.zeros((128, 200), np.float32)
    pidx = np.arange(128)
    node = (64 * (pidx % 16))[:, None] + np.arange(64)[None, :]   # [p, s*8+c]
    for j in range(3):
        clsT[:, j * 64:(j + 1) * 64] = cls_w[j][node]
    clsT[pidx, 192 + pidx // 16] = 1.0

    rows_per_core = BS * NN // N_CORES
    in_maps = [
        {"x": np.ascontiguousarray(xf[i * rows_per_core:(i + 1) * rows_per_core]),
         "clswt": clsT}
        for i in range(N_CORES)
    ]
    res = run_bass_kernel_spmd(nc, in_maps, list(range(N_CORES)), trace=trace,
                               **(trace_kwargs or {}))
    out = np.concatenate([np.asarray(res.results[i]["out"]) for i in range(N_CORES)],
                         axis=0).astype(np.float32)
    return out, res


def kernel(**inputs):
    out, _ = _run(inputs, trace=False)
    return out
